# revision 1
# baseline (speedup 1.0000x reference)
"""Maxwell viscoelastic model (linear recurrence scan) on 8 Trainium2 NeuronCores.

Math (per trajectory, T timesteps):
    a_n = 1 - k*dt_n              (k = E/eta = 2)
    b_n = k*dt_n*eps_n
    gamma_n = a_n*gamma_{n-1} + b_n,  gamma_0 = 0
    sigma_n = (E_inf + E)*eps_n - E*gamma_n = 2.5*eps_n - 2*gamma_n

Kernel strategy: shard the batch (4096 trajectories) across 8 cores (512
each) — the recurrence is independent per trajectory, so pure data
parallelism.  Per core, 4 tiles of [128 partitions x 4096 timesteps], cut
into CH time-chunks that stream through a software pipeline.  The
recurrence runs on the DVE tensor_tensor_scan instruction:
    state = (data0 * state) + data1   per partition, along the free dim.
We scan g_n = a_n*g_{n-1} + (-E*b_n) so g = -E*gamma directly, then
sigma = (eps * 2.5) + g in one scalar_tensor_tensor op.

Engine split (the DVE's second SBUF read port is the one shared with
GpSimd, so every 2-input DVE op with both operands in SBUF locks GpSimd
out; routing one operand of each DVE op through PSUM frees that port):
  SYNC   loads xt chunks (HWDGE qSPDynamicHW ring)
  ACT    a = 1 - k*dt  -> PSUM, and issues output stores (qActDynamicHW)
  POOL   bneg = (dt * -E*k) * eps -> SBUF   (runs concurrently with DVE)
  DVE    scan(a[PSUM], bneg[SBUF]) -> g[PSUM]; sigma(eps[SBUF], g[PSUM])

Raw bass (no TileContext): the Tile scheduler attaches semaphore waits
directly to instructions and overflows the tiny ISA sync-wait budgets
(S2S2D2_STT takes a single wait; the tail Drain takes few). With raw bass
every wait is a standalone instruction and the pipeline is explicit.
DMA completion uses one semaphore per (buffer slot, chunk): two DMAs on
one ring can complete out of order, so a shared counter cannot tell which
transfer finished.
"""

import numpy as np

import concourse.bass as bass
import concourse.mybir as mybir
from concourse.bass_utils import run_bass_kernel_spmd

E = 2.0
ETA = 1.0
E_INFTY = 0.5
K = E / ETA                  # 2.0
NEG_EK = -(E * K)            # -4.0: scan data1 scale so the scan outputs -E*gamma
SIG_EPS = E_INFTY + E        # 2.5

N_CORES = 8
P = 128                      # SBUF partitions
CH = 4                       # time chunks per tile
XT_BUFS = 3                  # xt ring depth


def build_nc(b_shard: int, t_len: int) -> bass.Bass:
    nc = bass.Bass()
    x = nc.dram_tensor("x", [b_shard, t_len, 2], mybir.dt.float32, kind="ExternalInput")
    y = nc.dram_tensor("y", [b_shard, t_len], mybir.dt.float32, kind="ExternalOutput")
    n_tiles = b_shard // P
    assert n_tiles * P == b_shard
    assert t_len % CH == 0
    L = t_len // CH

    xr = x.rearrange("(n p) t c -> n p t c", p=P)   # [n_tiles, 128, T, 2]
    yr = y.rearrange("(n p) t -> n p t", p=P)       # [n_tiles, 128, T]
    f32 = mybir.dt.float32
    mult = mybir.AluOpType.mult
    add = mybir.AluOpType.add

    def cs(c):
        return slice(c * L, (c + 1) * L)

    with (
        nc.sbuf_tensor("xt0", [P, t_len, 2], f32) as xt0,
        nc.sbuf_tensor("xt1", [P, t_len, 2], f32) as xt1,
        nc.sbuf_tensor("xt2", [P, t_len, 2], f32) as xt2,
        nc.sbuf_tensor("bneg0", [P, L], f32) as bneg0,
        nc.sbuf_tensor("bneg1", [P, L], f32) as bneg1,
        nc.sbuf_tensor("e40", [P, L], f32) as e40,
        nc.sbuf_tensor("e41", [P, L], f32) as e41,
        nc.sbuf_tensor("sig0", [P, t_len], f32) as sig0,
        nc.sbuf_tensor("sig1", [P, t_len], f32) as sig1,
        nc.psum_tensor("pa0", [P, L], f32) as pa0,
        nc.psum_tensor("pa1", [P, L], f32) as pa1,
        nc.psum_tensor("pg0", [P, L], f32) as pg0,
        nc.psum_tensor("pg1", [P, L], f32) as pg1,
        nc.semaphore("act_a") as act_a,        # +1 per a chunk (ACT)
        nc.semaphore("act_e") as act_e,        # +1 per e4 chunk (ACT)
        nc.semaphore("pool_seq") as pool_seq,  # +1 per POOL instruction
        nc.semaphore("dve_seq") as dve_seq,    # +1 per DVE instruction
        nc.Block(no_gpsimd_drain=True) as block,
    ):
        sem_in = [
            [nc.alloc_semaphore(f"in{s}_{c}") for c in range(CH)]
            for s in range(XT_BUFS)
        ]
        sem_out = [[nc.alloc_semaphore(f"out{s}_{c}") for c in range(CH)] for s in range(2)]
        xt = [xt0, xt1, xt2]
        bneg = [bneg0, bneg1]
        e4 = [e40, e41]
        sig = [sig0, sig1]
        pa = [pa0, pa1]
        pg = [pg0, pg1]
        # q = CH*i + c. DVE: 2 instrs per chunk (scan -> 2q+1, sigma -> 2q+2).
        # POOL: 1 instr per chunk (bneg -> q+1). ACT: 1 a per chunk (act_a -> q+1).

        @block.sync
        def _(sync):
            for i in range(n_tiles):
                for c in range(CH):
                    if i >= XT_BUFS:
                        # xt slot chunk reuse: sigma(i-XT_BUFS, c) transitively
                        # implies every reader of that chunk finished.
                        sync.wait_ge(dve_seq, 2 * (CH * (i - XT_BUFS) + c) + 2)
                    sync.dma_start(
                        xt[i % XT_BUFS][:, cs(c), :], xr[i][:, cs(c), :]
                    ).then_inc(sem_in[i % XT_BUFS][c], 16)

        @block.gpsimd
        def _(gpsimd):
            for i in range(n_tiles):
                for c in range(CH):
                    q = CH * i + c
                    dtv = xt[i % XT_BUFS][:, cs(c), 1]
                    gpsimd.wait_ge(sem_in[i % XT_BUFS][c], 16 * (i // XT_BUFS + 1))
                    gpsimd.wait_ge(act_e, q + 1)   # e4(q) ready
                    if q >= 2:
                        # bneg slot WAR: scan(q-2) was the last reader.
                        gpsimd.wait_ge(dve_seq, 2 * (q - 2) + 1)
                    # bneg = dt * (-E*K * eps)   (TensorScalarPtr is not legal
                    # on Pool, so the -E*K scale rides on ACT's e4 pass)
                    gpsimd.tensor_tensor(
                        bneg[q % 2][:], dtv, e4[q % 2][:], mult,
                    ).then_inc(pool_seq, 1)

        @block.scalar
        def _(scalar):
            def store(k):
                i, c = divmod(k, CH)
                scalar.wait_ge(dve_seq, 2 * k + 2)   # sigma(k) complete
                scalar.dma_start(
                    yr[i][:, cs(c)], sig[i % 2][:, cs(c)]
                ).then_inc(sem_out[i % 2][c], 16)

            for i in range(n_tiles):
                for c in range(CH):
                    q = CH * i + c
                    scalar.wait_ge(sem_in[i % XT_BUFS][c], 16 * (i // XT_BUFS + 1))
                    if q >= 2:
                        # a slot WAR: scan(q-2) read it.
                        scalar.wait_ge(dve_seq, 2 * (q - 2) + 1)
                    # a = Copy(dt * -K + 1) -> PSUM
                    scalar.activation(
                        pa[q % 2][:], xt[i % XT_BUFS][:, cs(c), 1],
                        mybir.ActivationFunctionType.Copy,
                        bias=1.0, scale=-K,
                    ).then_inc(act_a, 1)
                    if q >= 2:
                        # e4 slot WAR: bneg(q-2) read it.
                        scalar.wait_ge(pool_seq, q - 1)
                    # e4 = Copy(eps * -E*K) -> SBUF (feeds POOL's bneg)
                    scalar.activation(
                        e4[q % 2][:], xt[i % XT_BUFS][:, cs(c), 0],
                        mybir.ActivationFunctionType.Copy,
                        bias=0.0, scale=NEG_EK,
                    ).then_inc(act_e, 1)
                    if q >= 1:
                        store(q - 1)
            store(CH * n_tiles - 1)
            for c in range(CH):
                scalar.wait_ge(sem_out[0][c], 16 * ((n_tiles + 1) // 2))
                if n_tiles >= 2:
                    scalar.wait_ge(sem_out[1][c], 16 * (n_tiles // 2))

        @block.vector
        def _(vector):
            for i in range(n_tiles):
                for c in range(CH):
                    q = CH * i + c
                    eps = xt[i % XT_BUFS][:, cs(c), 0]
                    vector.wait_ge(sem_in[i % XT_BUFS][c], 16 * (i // XT_BUFS + 1))
                    vector.wait_ge(act_a, q + 1)       # a(q) in PSUM
                    vector.wait_ge(pool_seq, q + 1)    # bneg(q) in SBUF
                    if q >= 1:
                        vector.wait_ge(dve_seq, 2 * q)  # sigma(q-1) complete
                    # g_n = a_n*g_{n-1} + bneg_n  ->  g = -E*gamma
                    # Chain across chunks: initial = last element of the
                    # previous chunk's g; fresh 0 at each tile's chunk 0.
                    init = 0.0 if c == 0 else pg[(q - 1) % 2][:, L - 1:L]
                    vector.tensor_tensor_scan(
                        pg[q % 2][:], pa[q % 2][:], bneg[q % 2][:], init, mult, add,
                    ).then_inc(dve_seq, 1)
                    if i >= 2:
                        # sig slot chunk reuse: store(i-2, c) completed.
                        vector.wait_ge(sem_out[i % 2][c], 16 * ((i - 2) // 2 + 1))
                    vector.wait_ge(dve_seq, 2 * q + 1)   # scan complete
                    # sigma = (eps * 2.5) + g
                    vector.scalar_tensor_tensor(
                        sig[i % 2][:, cs(c)], eps, SIG_EPS, pg[q % 2][:], mult, add,
                    ).then_inc(dve_seq, 1)

    return nc


_NC_CACHE: dict = {}


def _get_nc(b_shard: int, t_len: int) -> bass.Bass:
    key = (b_shard, t_len)
    if key not in _NC_CACHE:
        _NC_CACHE[key] = build_nc(b_shard, t_len)
    return _NC_CACHE[key]


def run(x: np.ndarray, trace: bool = False):
    """Run the sharded kernel; returns (full_output, BassKernelResults)."""
    b, t_len, c = x.shape
    assert c == 2 and b % N_CORES == 0
    b_shard = b // N_CORES
    x = np.ascontiguousarray(np.asarray(x, dtype=np.float32))
    shards = x.reshape(N_CORES, b_shard, t_len, 2)
    in_maps = [{"x": shards[i]} for i in range(N_CORES)]
    res = run_bass_kernel_spmd(
        _get_nc(b_shard, t_len), in_maps,
        core_ids=list(range(N_CORES)), trace=trace,
    )
    out = np.concatenate([r["y"] for r in res.results], axis=0)
    return out.reshape(b, t_len, 1), res


def kernel(x: np.ndarray) -> np.ndarray:
    out, _ = run(x, trace=False)
    return out

